# revision 1
# baseline (speedup 1.0000x reference)
"""Self-contained Trainium2 Bass kernel for nn_AttentionBlock
(B=2, N=2048, D=512, H=8, MLP 2x).

kernel(**inputs) takes the FULL unsharded inputs (as produced by
setup_inputs) and returns the FULL (2, 2048, 512) output.

Sharding: 2-way data-parallel over batch x 4-way parallel over query-token
slices (8 cores, no collectives).  Each core computes K/V for its whole
batch and attention + MLP for its 512-token slice; the host stitches.
See build_nc's docstring for the kernel-internal strategy.
"""

from contextlib import ExitStack

import numpy as np

import concourse.bass as bass
import concourse.mybir as mybir
import concourse.tile as tile

_WSPLIT_UID = [0]


def _finalize(nc, max_waits=1):
    """Split multi-sem-wait instructions onto single-wait NoOp carriers
    (the walrus build in this container accepts one wait per instruction)."""
    for f in nc.m.functions:
        for bb in f.blocks:
            insts = bb.instructions
            out = []
            changed = False
            for inst in insts:
                si = inst.sync_info
                waits = list(si.on_wait) if (si and si.on_wait) else []
                if len(waits) > max_waits:
                    changed = True
                    for w in waits[:-max_waits]:
                        _WSPLIT_UID[0] += 1
                        nop = mybir.InstNoOp(
                            name=f"I-wsplit-{_WSPLIT_UID[0]}",
                            ins=[], outs=[], engine=inst.engine,
                        )
                        nop.sync_info = mybir.SyncInfo(on_wait=[w],
                                                       on_update=[])
                        out.append(nop)
                    si.on_wait = waits[-max_waits:]
                out.append(inst)
            if changed:
                bb.instructions = out
    return nc

BF16 = mybir.dt.bfloat16
F32 = mybir.dt.float32
F32R = mybir.dt.float32r
AF = mybir.ActivationFunctionType
OP = mybir.AluOpType

P = 128
B, N, D, H = 2, 2048, 512, 8
HD = D // H          # 64
TC = 512             # tokens per core
DM = 2 * D           # 1024 mlp hidden
KC = D // P          # 4 chunks of the 512 feature dim
NT = N // 512        # 4 column tiles of 512 over the 2048 kv tokens
JC = N // P          # 16 token chunks of 128 over kv tokens
MC1 = DM // P        # 8 chunks of mlp hidden
EPS = 1e-5


def dram_bcast_src(dram_ap, nparts):
    """AP re-reading a [1, n] DRAM row on `nparts` partitions (DMA src)."""
    return bass.AP(
        tensor=dram_ap.tensor,
        offset=dram_ap.offset,
        ap=[[0, nparts]] + [list(x) for x in dram_ap.ap[1:]],
    )


def build_nc(st_bufs=7, do_finalize=True):
    nc = bass.Bass()
    yT = nc.dram_tensor("yT", [D, N], F32, kind="ExternalInput")
    Wq = nc.dram_tensor("Wq", [D, D], F32, kind="ExternalInput")
    Wk = nc.dram_tensor("Wk", [D, D], F32, kind="ExternalInput")
    Wv = nc.dram_tensor("Wv", [D, D], F32, kind="ExternalInput")
    Wo = nc.dram_tensor("Wo", [D, D], F32, kind="ExternalInput")
    W1 = nc.dram_tensor("W1", [D, DM], F32, kind="ExternalInput")
    W2 = nc.dram_tensor("W2", [DM, D], F32, kind="ExternalInput")
    crow = nc.dram_tensor("crow", [4, D], F32, kind="ExternalInput")
    ccol = nc.dram_tensor("ccol", [P, 16], F32, kind="ExternalInput")
    outT = nc.dram_tensor("outT", [D, TC], F32, kind="ExternalOutput")

    BQ, NSQ, NSK, NSV = 0, 1, 2, 3  # crow rows

    def load_round(pool, stage, name, src, shape, eng=None):
        eng = eng or nc.sync
        tr = pool.tile(shape, F32R, name=name, tag=name)
        ncols = shape[-1] * (shape[-2] if len(shape) == 3 else 1)
        npieces = max(1, ncols // 2048)
        w = shape[-1] // npieces
        for i in range(npieces):
            cs = slice(i * w, (i + 1) * w)
            stg = stage.tile(shape[:-1] + [w], F32, name=f"{name}_s{i}",
                             tag="stage")
            eng.dma_start(out=stg[:], in_=src[..., cs])
            nc.gpsimd.tensor_copy(out=tr[..., cs], in_=stg[:])
        return tr

    def stat_sums(pstat, sqp, ones_div, xr, x_f32, mean_dst, msq_dst, kslice):
        """PE ones-reductions for one 512-token slice."""
        pm = pstat.tile([1, 512], F32, name="pm", tag="pmps")
        for k in range(KC):
            nc.tensor.matmul(pm[:], ones_div[:], xr[:, k, kslice],
                             start=(k == 0), stop=(k == KC - 1))
        nc.vector.tensor_copy(out=mean_dst, in_=pm[:])
        ps = pstat.tile([1, 512], F32, name="ps", tag="pmps")
        sq = sqp.tile([P, KC, 512], F32R, name="sq", tag="sq")
        nc.scalar.activation(out=sq[:], in_=x_f32[:, :, kslice],
                             func=AF.Square)
        for k in range(KC):
            nc.tensor.matmul(ps[:], ones_div[:], sq[:, k, :],
                             start=(k == 0), stop=(k == KC - 1))
        nc.vector.tensor_copy(out=msq_dst, in_=ps[:])

    def row_math(eps1, mean_row, sd_row, rstd_row, rs, ss=None):
        """var->sd->rstd on a row slice (msq pre-accumulated in rstd)."""
        ss = rs if ss is None else ss
        mean_f = mean_row.bitcast(F32)
        sd_f = sd_row.bitcast(F32)
        rstd_f = rstd_row.bitcast(F32)
        nc.vector.tensor_tensor(out=sd_row[:, ss], in0=mean_f[:, rs],
                                in1=mean_f[:, rs], op=OP.mult)
        nc.vector.tensor_tensor(out=rstd_row[:, rs], in0=rstd_f[:, rs],
                                in1=sd_f[:, ss], op=OP.subtract)
        nc.scalar.activation(out=sd_row[:, ss], in_=rstd_f[:, rs],
                             func=AF.Sqrt, bias=eps1[:])
        nc.vector.reciprocal(out=rstd_row[:, rs], in_=sd_f[:, ss])

    with tile.TileContext(nc, pool_alloc_mode="queue") as tc:
        with (
            tc.tile_pool(name="const", bufs=1) as const,
            tc.tile_pool(name="dpool", bufs=2, space="DRAM") as dpool,
            tc.tile_pool(name="stage", bufs=1) as stage,
            tc.tile_pool(name="rtp", bufs=1) as rtp,
            tc.tile_pool(name="y2p", bufs=1) as y2p,
        ):
            # ---- constants ----
            ident = const.tile([1, 1], F32)
            nc.vector.memset(ident[:], 1.0)
            eps1 = const.tile([1, 1], F32)
            nc.vector.memset(eps1[:], EPS)
            ones_f = const.tile([P, 1], F32)
            nc.vector.memset(ones_f[:], 1.0 / D)
            ones_div = const.tile([P, 1], F32R)
            nc.gpsimd.tensor_copy(out=ones_div[:], in_=ones_f[:])
            onec_f = const.tile([P, 1], F32)
            nc.vector.memset(onec_f[:], 1.0)
            onerow_f = const.tile([1, P], F32)
            nc.vector.memset(onerow_f[:], 1.0)
            ones_row = const.tile([1, P], F32R, name="ones_row")
            nc.gpsimd.tensor_copy(out=ones_row[:], in_=onerow_f[:])
            crow_r = load_round(const, stage, "crow_r",
                                crow.rearrange("(o r) d -> o r d", o=1),
                                [1, 4, D])
            ccol_sb = const.tile([P, 16], F32)
            nc.sync.dma_start(out=ccol_sb[:], in_=ccol[:])

            RT = rtp.tile([P, KC, TC], F32R, name="RT")
            y2T = y2p.tile([P, KC, TC], F32R, name="y2T")

            xtp_stack = ExitStack()
            xtp = xtp_stack.enter_context(tc.tile_pool(name="xtp", bufs=1))
            kqv_stack = ExitStack()
            kqv = kqv_stack.enter_context(tc.tile_pool(name="kqv", bufs=1))
            KT = kqv.tile([P, KC, N], F32R, name="KT")
            QT = kqv.tile([P, KC, TC], F32R, name="QT")
            V_st = kqv.tile([P, JC, H, HD + 1], BF16, name="V_st")

            # ================= phase A =================
            a_stack = ExitStack()
            pha = a_stack.enter_context(tc.tile_pool(name="pha", bufs=1))
            sqp = a_stack.enter_context(tc.tile_pool(name="sqp", bufs=1))
            arp = a_stack.enter_context(tc.tile_pool(name="arp", bufs=4))
            rowsA = a_stack.enter_context(tc.tile_pool(name="rowsA", bufs=1))
            pstat_stack = ExitStack()
            pmmA = pstat_stack.enter_context(
                tc.tile_pool(name="pmmA", bufs=4, space="PSUM"))
            pstatA = pstat_stack.enter_context(
                tc.tile_pool(name="pstatA", bufs=2, space="PSUM"))

            stageA_stack = ExitStack()
            stageA = stageA_stack.enter_context(
                tc.tile_pool(name="stageA", bufs=3))
            xTr = load_round(xtp, stageA, "xT",
                             yT.rearrange("(o p) t -> p o t", p=P),
                             [P, KC, N])
            xT = xTr.bitcast(F32)
            # own tokens are columns 0:TC of the (host-rotated) batch
            qsbr = xTr[:, :, 0:TC]
            qsb = xT[:, :, 0:TC]
            Wv_sb = load_round(pha, stageA, "wv",
                               Wv.rearrange("(o p) n -> p o n", p=P),
                               [P, KC, D])
            Wk_sb = load_round(pha, stageA, "wk",
                               Wk.rearrange("(o p) n -> p o n", p=P),
                               [P, KC, D])
            Wq_sb = load_round(pha, stageA, "wq",
                               Wq.rearrange("(o p) n -> p o n", p=P),
                               [P, KC, D])
            stageA_stack.close()

            # ---- LN1 stats (per t-tile) + rstd transposes ----
            mean_row = rowsA.tile([1, N], F32R, name="mean_ln1")
            sd_row = rowsA.tile([1, 512], F32R, name="sd_ln1")
            rstd_row = rowsA.tile([1, N], F32, name="rstd_ln1")
            rstd_tok = pha.tile([P, JC], F32, name="rstd_tok")
            rstd_dr = dpool.tile([1, N], F32, name="rstd_dr", tag="rd")
            arep_ts = []
            nc.vector.tensor_copy(out=V_st[:, :, :, HD:HD + 1],
                                  in_=onec_f.to_broadcast((P, JC, H, 1)))
            for nt in range(NT):
                ts = slice(nt * 512, nt * 512 + 512)
                stat_sums(pstatA, sqp, ones_div, xTr, xT,
                          mean_row[:, ts], rstd_row[:, ts], ts)
                row_math(eps1, mean_row, sd_row, rstd_row, ts, slice(0, 512))
                nc.sync.dma_start(out=rstd_dr[:, ts], in_=rstd_row[:, ts])
                arep_t = arp.tile([P, 512], F32, name="arep_t", tag="arep")
                nc.sync.dma_start(out=arep_t[:],
                                  in_=dram_bcast_src(rstd_dr[:, ts], P))
                arep_ts.append(arep_t)
                for jc in range(nt * 4, nt * 4 + 4):
                    pt = pstatA.tile([P, 1], F32, name="pt", tag="pt")
                    nc.tensor.transpose(
                        pt[:], rstd_row[:, jc * P:(jc + 1) * P], ident[:])
                    nc.vector.tensor_copy(out=rstd_tok[:, jc:jc + 1],
                                          in_=pt[:])
                if nt == 0:
                    # capture tile-0 sd for the later Q projection
                    sdq_row = rowsA.tile([1, TC], F32R, name="sdq")
                    nc.vector.tensor_copy(out=sdq_row[:],
                                          in_=sd_row[:, 0:512])
                # V projection for this tile's 4 token chunks
                for jc in range(nt * 4, nt * 4 + 4):
                    js = slice(jc * P, jc * P + P)
                    pv = pmmA.tile([P, 512], F32, name="pv", tag="pk")
                    for k in range(KC):
                        nc.tensor.matmul(pv[:], xTr[:, k, js], Wv_sb[:, k, :],
                                         start=(k == 0), stop=False)
                    nc.tensor.matmul(pv[:], mean_row[:, js],
                                     crow_r[:, NSV, :],
                                     start=False, stop=True)
                    nc.scalar.activation(
                        out=V_st[:, jc, :, 0:HD],
                        in_=pv.rearrange("p (h c) -> p h c", h=H),
                        func=AF.Copy, scale=rstd_tok[:, jc:jc + 1])
            # Q projection: q tokens are exactly tile 0 (host-rotated)
            for m in range(KC):
                ms = slice(m * P, m * P + P)
                pq = pmmA.tile([P, 512], F32, name="pq", tag="pk")
                for k in range(KC):
                    nc.tensor.matmul(pq[:], Wq_sb[:, k, ms], qsbr[:, k, :],
                                     start=(k == 0), stop=False)
                nc.tensor.matmul(pq[:], crow_r[:, NSQ, ms],
                                 mean_row[:, 0:TC], start=False, stop=False)
                nc.tensor.matmul(pq[:], crow_r[:, BQ, ms], sdq_row[:],
                                 start=False, stop=True)
                nc.vector.tensor_tensor(out=QT[:, m, :], in0=pq[:],
                                        in1=arep_ts[0][:], op=OP.mult)

            pstat_stack.close()

            # ========= K projection interleaved with attention =========
            with (
                tc.tile_pool(name="pkB", bufs=2, space="PSUM") as pkB,
                tc.tile_pool(name="stp", bufs=st_bufs) as stp,
                tc.tile_pool(name="rpool", bufs=2) as rpool,
                tc.tile_pool(name="pss", bufs=2, space="PSUM") as pss,
                tc.tile_pool(name="psr", bufs=2, space="PSUM") as psr,
            ):
                for m in range(KC):
                    ms = slice(m * P, m * P + P)
                    for nt in range(NT):
                        ts = slice(nt * 512, nt * 512 + 512)
                        pk = pkB.tile([P, 512], F32, name="pk", tag="pk")
                        for k in range(KC):
                            nc.tensor.matmul(pk[:], Wk_sb[:, k, ms],
                                             xTr[:, k, ts],
                                             start=(k == 0), stop=False)
                        nc.tensor.matmul(pk[:], crow_r[:, NSK, ms],
                                         mean_row[:, ts],
                                         start=False, stop=True)
                        nc.vector.tensor_tensor(out=KT[:, m, ts], in0=pk[:],
                                                in1=arep_ts[nt][:],
                                                op=OP.mult)
                    # even/odd heads' scores interleaved: base-partition
                    # 0/64 row groups run concurrently on the PE array
                    st_all = {0: [], 1: []}
                    for jp in range(JC // 2):
                        pscs = {}
                        for r in range(2):
                            hs = slice(r * HD, r * HD + HD)
                            psc = pss.tile([P, 1024], F32, name="psc",
                                           tag="psc")
                            for half in range(2):
                                jc = jp * 2 + half
                                js = slice(jc * P, jc * P + P)
                                nc.tensor.matmul(
                                    psc[:, half * 512:half * 512 + 512],
                                    KT[hs, m, js], QT[hs, m, :],
                                    start=True, stop=True)
                            pscs[r] = psc
                        for r in range(2):
                            st = stp.tile([P, 1024], BF16, name="st",
                                          tag="st")
                            nc.scalar.activation(
                                out=st[:], in_=pscs[r][:], func=AF.Exp,
                                scale=float(1.0 / np.sqrt(HD)))
                            st_all[r].append(st)
                    for r in range(2):
                        h = 2 * m + r
                        hs = slice(r * HD, r * HD + HD)
                        st_tiles = st_all[r]
                        pr = psr.tile([HD + 1, 512], F32, name="pr",
                                      tag="pr")
                        for jc in range(JC):
                            nc.tensor.matmul(
                                pr[:], V_st[:, jc, h, :],
                                st_tiles[jc // 2][:, (jc % 2) * 512:
                                                  (jc % 2) * 512 + 512],
                                start=(jc == 0), stop=(jc == JC - 1))
                        rs_row = rpool.tile([1, TC], F32, name="rs_row",
                                            tag="rs")
                        nc.vector.reciprocal(out=rs_row[:],
                                             in_=pr[HD:HD + 1, :])
                        rs_dr = dpool.tile([1, TC], F32, name="rs_dr",
                                           tag="rsd")
                        nc.sync.dma_start(out=rs_dr[:], in_=rs_row[:])
                        rrep = rpool.tile([HD, TC], F32, name="rrep",
                                          tag="rrep")
                        nc.sync.dma_start(out=rrep[:],
                                          in_=dram_bcast_src(rs_dr[:], HD))
                        nc.vector.tensor_tensor(out=RT[hs, m, :],
                                                in0=pr[0:HD, :],
                                                in1=rrep[:], op=OP.mult)

            a_stack.close()

            # ================= phase C =================
            with (
                tc.tile_pool(name="phc", bufs=1) as phc,
                tc.tile_pool(name="sqc", bufs=1) as sqc,
                tc.tile_pool(name="rowsC", bufs=1) as rowsC,
                tc.tile_pool(name="pmmC", bufs=4, space="PSUM") as pmmC,
            ):
                pstatC_stack = ExitStack()
                pstatC = pstatC_stack.enter_context(
                    tc.tile_pool(name="pstatC", bufs=2, space="PSUM"))
                def load_round_k(name, src, shape):
                    # piece along the contraction-chunk dim so consumers
                    # start as soon as their chunk lands
                    tr = phc.tile(shape, F32R, name=name, tag=name)
                    for k in range(shape[1]):
                        stg = stage.tile([shape[0], 1, shape[2]], F32,
                                         name=f"{name}_k{k}", tag="stage")
                        nc.gpsimd.dma_start(out=stg[:],
                                            in_=src[:, k:k + 1, :])
                        nc.gpsimd.tensor_copy(out=tr[:, k:k + 1, :],
                                              in_=stg[:])
                    return tr

                Wo_sb = load_round_k("wo",
                                     Wo.rearrange("(o p) n -> p o n", p=P),
                                     [P, KC, D])
                W1_sb = load_round_k("w1",
                                     W1.rearrange("(o p) n -> p o n", p=P),
                                     [P, KC, DM])
                W2_sb = load_round_k("w2",
                                     W2.rearrange("(o p) n -> p o n", p=P),
                                     [P, MC1, D])

                # output projection + residual + bo, LN2 stat terms
                # accumulated per chunk as soon as each drain lands
                y2f = y2T.bitcast(F32)
                mean2_row = rowsC.tile([1, TC], F32R, name="mean2")
                sd2_row = rowsC.tile([1, TC], F32R, name="sd2")
                rstd2_row = rowsC.tile([1, TC], F32R, name="rstd2")
                pm2 = pstatC.tile([1, 512], F32, name="pm", tag="pmps")
                ps2 = pstatC.tile([1, 512], F32, name="ps", tag="pmps")
                po_s = [pmmC.tile([P, 512], F32, name=f"po{m}", tag="po")
                        for m in range(KC)]
                for k in range(KC):
                    for m in range(KC):
                        ms = slice(m * P, m * P + P)
                        nc.tensor.matmul(po_s[m][:], Wo_sb[:, k, ms],
                                         RT[:, k, :],
                                         start=(k == 0), stop=(k == KC - 1))
                for m in range(KC):
                    nc.vector.scalar_tensor_tensor(
                        out=y2T[:, m, :], in0=po_s[m][:],
                        scalar=ccol_sb[:, m:m + 1],
                        in1=qsb[:, m, :], op0=OP.add, op1=OP.add)
                    nc.tensor.matmul(pm2[:], ones_div[:], y2T[:, m, :],
                                     start=(m == 0), stop=(m == KC - 1))
                    sq = sqc.tile([P, 512], F32R, name="sq", tag="sq")
                    nc.scalar.activation(out=sq[:], in_=y2f[:, m, :],
                                         func=AF.Square)
                    nc.tensor.matmul(ps2[:], ones_div[:], sq[:],
                                     start=(m == 0), stop=(m == KC - 1))
                nc.vector.tensor_copy(out=mean2_row[:], in_=pm2[:])
                nc.vector.tensor_copy(out=rstd2_row[:], in_=ps2[:])
                mean2_f = mean2_row.bitcast(F32)
                sd2_f = sd2_row.bitcast(F32)
                rstd2_f = rstd2_row.bitcast(F32)
                nc.vector.tensor_tensor(out=sd2_row[:], in0=mean2_f[:],
                                        in1=mean2_f[:], op=OP.mult)
                nc.vector.tensor_tensor(out=rstd2_row[:], in0=rstd2_f[:],
                                        in1=sd2_f[:], op=OP.subtract)
                nc.scalar.activation(out=sd2_row[:], in_=rstd2_f[:],
                                     func=AF.Sqrt, bias=eps1[:])
                with nc.allow_low_precision(reason="f32r == f32 bits"):
                    nc.vector.reciprocal(out=rstd2_row[:], in_=sd2_f[:])

                pmrep = pstatC.tile([P, 512], F32, name="pmrep", tag="rep")
                nc.tensor.matmul(pmrep[:], ones_row[:], mean2_row[:],
                                 start=True, stop=True)
                prrep = pstatC.tile([P, 512], F32, name="prrep", tag="rep")
                nc.tensor.matmul(prrep[:], ones_row[:], rstd2_row[:],
                                 start=True, stop=True)

                z2T = phc.tile([P, KC, TC], F32R, name="z2T")
                z2f = z2T.bitcast(F32)
                for k in range(KC):
                    nc.vector.tensor_tensor(out=z2T[:, k, :], in0=y2f[:, k, :],
                                            in1=pmrep[:], op=OP.subtract)
                    nc.vector.tensor_tensor(out=z2T[:, k, :], in0=z2f[:, k, :],
                                            in1=prrep[:], op=OP.mult)

                pstatC_stack.close()
                # MLP: h2 accumulates k-major as h1 chunks complete
                h1T = phc.tile([P, MC1, TC], F32R, name="h1T")
                ph2_stack = ExitStack()
                ph2 = ph2_stack.enter_context(
                    tc.tile_pool(name="ph2", bufs=1, space="PSUM"))
                p2s = [ph2.tile([P, 512], F32, name=f"p2_{m}", tag=f"p2_{m}")
                       for m in range(KC)]
                for k in range(MC1):
                    ks = slice(k * P, k * P + P)
                    p1 = pmmC.tile([P, 512], F32, name="p1", tag="po")
                    for kk in range(KC):
                        nc.tensor.matmul(p1[:], W1_sb[:, kk, ks],
                                         z2T[:, kk, :],
                                         start=(kk == 0), stop=(kk == KC - 1))
                    nc.scalar.activation(out=h1T[:, k, :], in_=p1[:],
                                         func=AF.Gelu,
                                         bias=ccol_sb[:, 4 + k:5 + k])
                    for m in range(KC):
                        ms = slice(m * P, m * P + P)
                        nc.tensor.matmul(p2s[m][:], W2_sb[:, k, ms],
                                         h1T[:, k, :],
                                         start=(k == 0), stop=(k == MC1 - 1))

                out_sb = phc.tile([P, KC, TC], F32, name="out_sb")
                outT_r = outT.rearrange("(o p) t -> p o t", p=P)
                for m in range(KC):
                    nc.vector.scalar_tensor_tensor(
                        out=out_sb[:, m, :], in0=p2s[m][:],
                        scalar=ccol_sb[:, 12 + m:13 + m],
                        in1=y2f[:, m, :], op0=OP.add, op1=OP.add)
                    nc.sync.dma_start(out=outT_r[:, m, :],
                                      in_=out_sb[:, m, :])
                ph2_stack.close()

            kqv_stack.close()
            xtp_stack.close()

    return _finalize(nc) if do_finalize else nc


def prep_inputs(y, Wq, bq, Wk, bk, Wv, bv, Wo, bo, ln1_g, ln1_b, ln2_g, ln2_b,
                W1, b1, W2, b2):
    """Host-side weight folding + per-core input maps."""
    f = np.float32
    Wq_ = (Wq * ln1_g[:, None]).astype(f)
    Wk_ = (Wk * ln1_g[:, None]).astype(f)
    Wv_ = (Wv * ln1_g[:, None]).astype(f)
    bq_ = (ln1_b @ Wq + bq).astype(f)
    bv_ = (ln1_b @ Wv + bv).astype(f)
    bo_ = (bv_ @ Wo + bo).astype(f)
    W1_ = (W1 * ln2_g[:, None]).astype(f)
    b1_ = (ln2_b @ W1 + b1).astype(f)

    crow = np.stack([bq_, -Wq_.sum(0), -Wk_.sum(0), -Wv_.sum(0)]).astype(f)
    ccol = np.concatenate([
        bo_.reshape(4, P).T, b1_.reshape(8, P).T,
        np.asarray(b2, f).reshape(4, P).T,
    ], axis=1).astype(f)

    shared = {
        "Wq": np.ascontiguousarray(Wq_), "Wk": np.ascontiguousarray(Wk_),
        "Wv": np.ascontiguousarray(Wv_),
        "Wo": np.ascontiguousarray(Wo, dtype=f),
        "W1": np.ascontiguousarray(W1_),
        "W2": np.ascontiguousarray(W2, dtype=f),
        "crow": crow, "ccol": ccol,
    }
    in_maps = []
    for c in range(8):
        b, s = divmod(c, 4)
        ts = s * TC
        yTm = np.asarray(y, f)[b].T
        yrot = np.ascontiguousarray(np.roll(yTm, -ts, axis=1))
        in_maps.append({"yT": yrot, **shared})
    return in_maps


def gather_output(results):
    out = np.empty((B, N, D), np.float32)
    for c in range(8):
        b, s = divmod(c, 4)
        out[b, s * TC:(s + 1) * TC, :] = results[c]["outT"].T
    return out


_NC_CACHE = {}


def kernel(**inputs):
    """Full-input entry point: shard, run on 8 NeuronCores, gather."""
    from concourse.bass_utils import run_bass_kernel_spmd

    in_maps = prep_inputs(**{k: np.asarray(v) for k, v in inputs.items()})
    if "nc" not in _NC_CACHE:
        _NC_CACHE["nc"] = build_nc()
    nc = _NC_CACHE["nc"]
    res = run_bass_kernel_spmd(nc, in_maps, core_ids=list(range(8)))
    return gather_output(res.results)



# revision 6
# speedup vs baseline: 1.3435x; 1.3435x over previous
"""Self-contained Trainium2 Bass kernel for nn_AttentionBlock
(B=2, N=2048, D=512, H=8, MLP 2x).

kernel(**inputs) takes the FULL unsharded inputs (as produced by
setup_inputs) and returns the FULL (2, 2048, 512) output.

Sharding: 2-way data-parallel over batch x 4-way parallel over query-token
slices (8 cores, no collectives).  Each core computes K/V for its whole
batch and attention + MLP for its 512-token slice; the host stitches.

v2: fp8(e4m3) DoubleRow matmuls for all deep GEMMs (projections, AV,
MLP) and zero-padded DoubleRow for the 64-deep attention scores;
weights/activations quantized host-side; PE-based row broadcasts;
rstd via exp(-0.5*ln(var)) keeps the Act engine on one function table.
"""

from contextlib import ExitStack

import numpy as np
import ml_dtypes

import concourse.bass as bass
import concourse.mybir as mybir
import concourse.tile as tile

_WSPLIT_UID = [0]


def _finalize(nc, max_waits=1):
    """Split multi-sem-wait instructions onto single-wait NoOp carriers
    (the walrus build in this container accepts one wait per instruction)."""
    for f in nc.m.functions:
        for bb in f.blocks:
            insts = bb.instructions
            out = []
            changed = False
            for inst in insts:
                si = inst.sync_info
                waits = list(si.on_wait) if (si and si.on_wait) else []
                if len(waits) > max_waits:
                    changed = True
                    for w in waits[:-max_waits]:
                        _WSPLIT_UID[0] += 1
                        nop = mybir.InstNoOp(
                            name=f"I-wsplit-{_WSPLIT_UID[0]}",
                            ins=[], outs=[], engine=inst.engine,
                        )
                        nop.sync_info = mybir.SyncInfo(on_wait=[w],
                                                       on_update=[])
                        out.append(nop)
                    si.on_wait = waits[-max_waits:]
                out.append(inst)
            if changed:
                bb.instructions = out
    return nc

BF16 = mybir.dt.bfloat16
F32 = mybir.dt.float32
F32R = mybir.dt.float32r
FP8 = mybir.dt.float8e4
AF = mybir.ActivationFunctionType
OP = mybir.AluOpType
DR = mybir.MatmulPerfMode.DoubleRow

P = 128
B, N, D, H = 2, 2048, 512, 8
HD = D // H          # 64
TC = 512             # tokens per core
DM = 2 * D           # 1024 mlp hidden
KC = D // P          # 4 chunks of the 512 feature dim
NT = N // 512        # 4 tiles of 512 over the 2048 kv tokens
JC = N // P          # 16 token chunks of 128 over kv tokens
MC1 = DM // P        # 8 chunks of mlp hidden
VW = HD + 1          # V row width (ones column for the softmax denom)
MB = N + P           # per head-pair block width in KT8 (incl. 128 pad)

BQ, NSQ, NSK, NSV = 0, 1, 2, 3  # crow rows


def build_nc(st_bufs=24, do_finalize=True):
    nc = bass.Bass()
    y8T = nc.dram_tensor("y8T", [D, N], FP8, kind="ExternalInput")
    yoT = nc.dram_tensor("yoT", [D, TC], F32, kind="ExternalInput")
    Wq8 = nc.dram_tensor("Wq8", [D, D], FP8, kind="ExternalInput")
    Wk8 = nc.dram_tensor("Wk8", [D, D], FP8, kind="ExternalInput")
    Wv8 = nc.dram_tensor("Wv8", [D, D], FP8, kind="ExternalInput")
    Wo8 = nc.dram_tensor("Wo8", [D, D], FP8, kind="ExternalInput")
    W18 = nc.dram_tensor("W18", [D, DM], FP8, kind="ExternalInput")
    W28 = nc.dram_tensor("W28", [DM, D], FP8, kind="ExternalInput")
    crow = nc.dram_tensor("crow", [4, D], F32, kind="ExternalInput")
    ccol = nc.dram_tensor("ccol", [P, 16], F32, kind="ExternalInput")
    outT = nc.dram_tensor("outT", [D, TC], F32, kind="ExternalOutput")

    y8r = y8T.rearrange("(o p) t -> p o t", p=P)
    yor = yoT.rearrange("(o p) t -> p o t", p=P)
    wq8r = Wq8.rearrange("(o p) n -> p o n", p=P)
    wk8r = Wk8.rearrange("(o p) n -> p o n", p=P)
    wv8r = Wv8.rearrange("(o p) n -> p o n", p=P)
    wo8r = Wo8.rearrange("(o p) n -> p o n", p=P)
    w18r = W18.rearrange("(o p) n -> p o n", p=P)
    w28r = W28.rearrange("(o p) n -> p o n", p=P)

    with tile.TileContext(nc, pool_alloc_mode="queue") as tc:
        with (
            tc.tile_pool(name="const", bufs=1) as const,
            tc.tile_pool(name="xp", bufs=1) as xp,
            tc.tile_pool(name="rows", bufs=1) as rows,
            tc.tile_pool(name="rtmp", bufs=4) as rtmp,
            tc.tile_pool(name="kqv", bufs=1) as kqv,
            tc.tile_pool(name="rt8p", bufs=1) as rt8p,
            tc.tile_pool(name="y2p", bufs=1) as y2p,
        ):
            # ---- constants ----
            ident = const.tile([1, 1], F32)
            nc.vector.memset(ident[:], 1.0)
            ones8 = const.tile([P, 2, 1], FP8, name="ones8")
            nc.vector.memset(ones8[:], 1.0)
            onec_r = const.tile([P, 1], F32R, name="onec_r")
            nc.vector.memset(onec_r[:], 1.0)
            ones_row = const.tile([1, P], F32R, name="ones_row")
            nc.vector.memset(ones_row[:], 1.0)
            e_row = const.tile([1, P], F32R, name="e_row")
            nc.vector.memset(e_row[:], 0.125)
            oned_row = const.tile([1, P], F32R, name="oned_row")
            nc.vector.memset(oned_row[:], 1.0 / 512.0)
            ccol_sb = const.tile([P, 16], F32)
            nc.sync.dma_start(out=ccol_sb[:], in_=ccol[:])
            crow_sb = const.tile([1, 4, D], F32, name="crow_sb")
            nc.sync.dma_start(out=crow_sb[:],
                              in_=crow.rearrange("(o r) d -> o r d", o=1))
            crow_r = crow_sb.bitcast(F32R)

            # ---- big SBUF tensors ----
            y8 = xp.tile([P, KC, N], FP8, name="y8")
            sq8 = xp.tile([P, KC, N], FP8, name="sq8")
            xo = xp.tile([P, KC, TC], F32, name="xo")
            Wq_s = xp.tile([P, KC, D], FP8, name="Wq_s")
            Wk_s = xp.tile([P, KC, D], FP8, name="Wk_s")
            Wv_s = xp.tile([P, KC, D], FP8, name="Wv_s")
            KT8 = kqv.tile([P, KC * MB], FP8, name="KT8")
            QT8 = kqv.tile([P, KC, 2, TC], FP8, name="QT8")
            V8 = kqv.tile([P, JC, H, VW], FP8, name="V8")
            RT8 = rt8p.tile([P, KC, TC], FP8, name="RT8")
            y2T = y2p.tile([P, KC, TC], F32, name="y2T")

            # input DMAs: y8 split in 4 token-pieces across SP/Pool queues
            for i in range(NT):
                ts = slice(i * 512, (i + 1) * 512)
                eng = nc.sync if i % 2 == 0 else nc.gpsimd
                eng.dma_start(out=y8[:, :, ts], in_=y8r[:, :, ts])
            nc.sync.dma_start(out=Wv_s[:], in_=wv8r[:])
            nc.sync.dma_start(out=Wk_s[:], in_=wk8r[:])
            nc.sync.dma_start(out=Wq_s[:], in_=wq8r[:])
            for i in range(2):
                cs = slice(i * 256, (i + 1) * 256)
                nc.gpsimd.dma_start(out=xo[:, :, cs], in_=yor[:, :, cs])

            # zero pads: QT8 z=1 blocks and KT8 per-m pad blocks
            with nc.allow_low_precision(reason="fp8 zeros"):
                nc.vector.memset(QT8[:, :, 1, :], 0.0)
                nc.vector.memset(
                    KT8[:].rearrange("p (o t) -> p o t", o=KC)[:, :, N:MB],
                    0.0)
                nc.vector.memset(V8[:, :, :, HD:VW], 1.0)

            # rows
            S_row = rows.tile([1, N], F32, name="S_row")
            rstd_row = rows.tile([1, N], F32, name="rstd_row")
            sd_row = rows.tile([1, TC], F32, name="sd_row")
            rstd_tok = rows.tile([P, JC], F32, name="rstd_tok")
            arep_sb = rows.tile([P, NT, 512], BF16, name="arep_sb")
            Sr = S_row.bitcast(F32R)

            a_stack = ExitStack()
            pstat = a_stack.enter_context(
                tc.tile_pool(name="pstat", bufs=2, space="PSUM"))
            parep = a_stack.enter_context(
                tc.tile_pool(name="parep", bufs=2, space="PSUM"))
            ppt = a_stack.enter_context(
                tc.tile_pool(name="ppt", bufs=2, space="PSUM"))

            # ================= LN1 stats (all 4 tiles) =================
            for nt in range(NT):
                ts = slice(nt * 512, nt * 512 + 512)
                sq_eng = nc.vector if nt < 2 else nc.gpsimd
                with nc.allow_low_precision(reason="fp8 x^2 for stats"):
                    sq_eng.tensor_tensor(out=sq8[:, :, ts], in0=y8[:, :, ts],
                                         in1=y8[:, :, ts], op=OP.mult)
                pm = pstat.tile([1, 512], F32, name="pm", tag="pm")
                for t in range(2):
                    nc.tensor.matmul(pm[:], ones8[:],
                                     y8[:, 2 * t:2 * t + 2, ts],
                                     start=(t == 0), stop=(t == 1),
                                     perf_mode=DR)
                ps = pstat.tile([1, 512], F32, name="ps", tag="pm")
                for t in range(2):
                    nc.tensor.matmul(ps[:], ones8[:],
                                     sq8[:, 2 * t:2 * t + 2, ts],
                                     start=(t == 0), stop=(t == 1),
                                     perf_mode=DR)
                nc.vector.tensor_copy(out=S_row[:, ts], in_=pm[:])
                t_row = rtmp.tile([1, 512], F32, name="t_row", tag="t_row")
                nc.vector.tensor_tensor(out=t_row[:], in0=S_row[:, ts],
                                        in1=S_row[:, ts], op=OP.mult)
                var_row = rtmp.tile([1, 512], F32, name="var_row",
                                    tag="var_row")
                nc.vector.scalar_tensor_tensor(
                    out=var_row[:], in0=t_row[:], scalar=-1.0 / 512.0,
                    in1=ps[:], op0=OP.mult, op1=OP.add)
                ln_row = rtmp.tile([1, 512], F32, name="ln_row", tag="ln_row")
                nc.scalar.activation(out=ln_row[:], in_=var_row[:],
                                     func=AF.Ln, scale=1.0 / 512.0)
                nc.scalar.activation(out=rstd_row[:, ts], in_=ln_row[:],
                                     func=AF.Exp, scale=-0.5)
                if nt == 0:
                    nc.scalar.activation(out=sd_row[:], in_=ln_row[:],
                                         func=AF.Exp, scale=0.5)
                # replicate rstd across partitions via PE, stash as bf16
                pa = parep.tile([P, 512], F32, name="pa", tag="pa")
                nc.tensor.matmul(pa[:], ones_row[:],
                                 rstd_row.bitcast(F32R)[:, ts],
                                 start=True, stop=True)
                with nc.allow_low_precision(reason="bf16 rstd replica"):
                    nc.gpsimd.tensor_copy(out=arep_sb[:, nt, :], in_=pa[:])
                # per-token rstd columns (for the V eviction scale)
                for jc in range(nt * 4, nt * 4 + 4):
                    pt = ppt.tile([P, 1], F32, name="pt", tag="pt")
                    nc.tensor.transpose(
                        pt[:], rstd_row[:, jc * P:(jc + 1) * P], ident[:])
                    nc.vector.tensor_copy(out=rstd_tok[:, jc:jc + 1],
                                          in_=pt[:])
            a_stack.close()

            # ================= Q projection (tile 0 tokens) =================
            with tc.tile_pool(name="pkv", bufs=2, space="PSUM") as pkv:
                for m in range(KC):
                    ms = slice(m * P, m * P + P)
                    pq = pkv.tile([P, 512], F32, name="pq", tag="pk")
                    for t in range(2):
                        nc.tensor.matmul(pq[:], Wq_s[:, 2 * t:2 * t + 2, ms],
                                         y8[:, 2 * t:2 * t + 2, 0:TC],
                                         start=(t == 0), stop=False,
                                         perf_mode=DR)
                    nc.tensor.matmul(pq[:], crow_r[:, NSQ, ms], Sr[:, 0:TC],
                                     start=False, stop=False)
                    nc.tensor.matmul(pq[:], crow_r[:, BQ, ms],
                                     sd_row.bitcast(F32R)[:],
                                     start=False, stop=True)
                    with nc.allow_low_precision(reason="fp8 evict"):
                        nc.vector.tensor_tensor(out=QT8[:, m, 0, :],
                                                in0=pq[:],
                                                in1=arep_sb[:, 0, :],
                                                op=OP.mult)

                # ========== K proj + scores (m-major), V, AV ==========
                with (
                    tc.tile_pool(name="pss", bufs=2, space="PSUM") as pss,
                    tc.tile_pool(name="stp", bufs=st_bufs) as stp,
                    tc.tile_pool(name="prp", bufs=2, space="PSUM") as prp,
                    tc.tile_pool(name="rsp", bufs=4) as rsp,
                ):
                    KTv = KT8[:].rearrange("p (o t) -> p o t", o=KC)

                    def kproj(m):
                        ms = slice(m * P, m * P + P)
                        for nt in range(NT):
                            ts = slice(nt * 512, nt * 512 + 512)
                            pk = pkv.tile([P, 512], F32, name="pk", tag="pk")
                            for t in range(2):
                                nc.tensor.matmul(
                                    pk[:], Wk_s[:, 2 * t:2 * t + 2, ms],
                                    y8[:, 2 * t:2 * t + 2, ts],
                                    start=(t == 0), stop=False, perf_mode=DR)
                            nc.tensor.matmul(pk[:], crow_r[:, NSK, ms],
                                             Sr[:, ts],
                                             start=False, stop=True)
                            eng = nc.vector if nt % 2 == 0 else nc.gpsimd
                            with nc.allow_low_precision(reason="fp8 evict"):
                                eng.tensor_tensor(out=KTv[:, m, ts],
                                                  in0=pk[:],
                                                  in1=arep_sb[:, nt, :],
                                                  op=OP.mult)

                    def scores(m):
                        for jp in range(JC // 2):
                            for r in range(2):
                                hs = slice(r * HD, r * HD + HD)
                                psc = pss.tile([P, 1024], F32, name="psc",
                                               tag="psc")
                                for half in range(2):
                                    jc = jp * 2 + half
                                    lhs = KT8[hs, m * MB + jc * P:
                                              m * MB + jc * P + 2 * P]
                                    lhs = lhs.rearrange("p (z t) -> p z t",
                                                        z=2)
                                    nc.tensor.matmul(
                                        psc[:, half * 512:half * 512 + 512],
                                        lhs, QT8[hs, m, :, :],
                                        start=True, stop=True, perf_mode=DR)
                                st = stp.tile([P, 1024], FP8, name="st",
                                              tag=f"st{r}")
                                with nc.allow_low_precision(reason="fp8 st"):
                                    nc.scalar.activation(
                                        out=st[:], in_=psc[:], func=AF.Exp,
                                        scale=1.0 / 512.0)
                                st_all[2 * m + r].append(st)

                    def vproj():
                        for jc in range(JC):
                            js = slice(jc * P, jc * P + P)
                            pv = pkv.tile([P, 512], F32, name="pv", tag="pk")
                            for t in range(2):
                                nc.tensor.matmul(
                                    pv[:], y8[:, 2 * t:2 * t + 2, js],
                                    Wv_s[:, 2 * t:2 * t + 2, :],
                                    start=(t == 0), stop=False, perf_mode=DR)
                            nc.tensor.matmul(pv[:], Sr[:, js],
                                             crow_r[:, NSV, :],
                                             start=False, stop=True)
                            with nc.allow_low_precision(reason="fp8 evict"):
                                nc.gpsimd.tensor_scalar(
                                    out=V8[:, jc, :, 0:HD],
                                    in0=pv.rearrange("p (h c) -> p h c", h=H),
                                    scalar1=rstd_tok[:, jc:jc + 1],
                                    scalar2=None, op0=OP.mult)

                    def av(h):
                        m, r = h // 2, h % 2
                        hs = slice(r * HD, r * HD + HD)
                        pr = prp.tile([VW, 512], F32, name="pr", tag="pr")
                        for jp in range(JC // 2):
                            nc.tensor.matmul(
                                pr[:], V8[:, 2 * jp:2 * jp + 2, h, :],
                                st_all[h][jp][:].rearrange(
                                    "p (z t) -> p z t", z=2),
                                start=(jp == 0), stop=(jp == JC // 2 - 1),
                                perf_mode=DR)
                        rs_row = rsp.tile([1, TC], F32, name="rs_row",
                                          tag="rs")
                        with nc.allow_low_precision(reason="recip"):
                            nc.vector.reciprocal(out=rs_row[:],
                                                 in_=pr[HD:HD + 1, :])
                        prr = prp.tile([HD, 512], F32, name="prr", tag="pr")
                        nc.tensor.matmul(prr[:], e_row[:, 0:HD],
                                         rs_row.bitcast(F32R)[:],
                                         start=True, stop=True)
                        with nc.allow_low_precision(reason="fp8 evict"):
                            nc.vector.tensor_tensor(out=RT8[hs, m, :],
                                                    in0=pr[0:HD, :],
                                                    in1=prr[:], op=OP.mult)

                    st_all = {h: [] for h in range(H)}
                    kproj(0)
                    scores(0)
                    vproj()
                    kproj(1)
                    scores(1)
                    av(0)
                    av(1)
                    kproj(2)
                    scores(2)
                    av(2)
                    av(3)
                    kproj(3)
                    scores(3)
                    for h in range(4, 8):
                        av(h)

            a_stack.close()

            # ================= phase C: O proj, LN2, MLP =================
            with (
                tc.tile_pool(name="phc", bufs=1) as phc,
                tc.tile_pool(name="pmmC", bufs=2, space="PSUM") as pmmC,
            ):
                Wo_s = phc.tile([P, KC, D], FP8, name="Wo_s")
                W1_s = phc.tile([P, KC, DM], FP8, name="W1_s")
                W2_s = phc.tile([P, MC1, D], FP8, name="W2_s")
                nc.sync.dma_start(out=Wo_s[:], in_=wo8r[:])
                nc.sync.dma_start(out=W1_s[:], in_=w18r[:])
                nc.sync.dma_start(out=W2_s[:], in_=w28r[:])

                pstatC_stack = ExitStack()
                pstatC = pstatC_stack.enter_context(
                    tc.tile_pool(name="pstatC", bufs=2, space="PSUM"))
                prepC = pstatC_stack.enter_context(
                    tc.tile_pool(name="prepC", bufs=2, space="PSUM"))

                pm2 = pstatC.tile([1, 512], F32, name="pm2", tag="pm2")
                ps2 = pstatC.tile([1, 512], F32, name="ps2", tag="pm2")
                y2r = y2T.bitcast(F32R)
                sq2s = []
                for m in range(KC):
                    ms = slice(m * P, m * P + P)
                    po = pmmC.tile([P, 512], F32, name="po", tag="po")
                    for t in range(2):
                        nc.tensor.matmul(po[:], Wo_s[:, 2 * t:2 * t + 2, ms],
                                         RT8[:, 2 * t:2 * t + 2, :],
                                         start=(t == 0), stop=(t == 1),
                                         perf_mode=DR)
                    nc.vector.scalar_tensor_tensor(
                        out=y2T[:, m, :], in0=po[:], scalar=0.0,
                        in1=xo[:, m, :], op0=OP.add, op1=OP.add)
                    nc.tensor.matmul(pm2[:], onec_r[:], y2r[:, m, :],
                                     start=(m == 0), stop=(m == KC - 1))
                    sq2 = rtmp.tile([P, 512], BF16, name="sq2",
                                    tag=f"sq2_{m}")
                    with nc.allow_low_precision(reason="bf16 y2^2"):
                        nc.vector.tensor_tensor(out=sq2[:], in0=y2T[:, m, :],
                                                in1=y2T[:, m, :], op=OP.mult)
                    sq2s.append(sq2)
                for m in range(KC):
                    nc.tensor.matmul(ps2[:], onec_r[:], sq2s[m][:],
                                     start=(m == 0), stop=(m == KC - 1))
                S2_row = rows.tile([1, TC], F32, name="S2_row")
                nc.vector.tensor_copy(out=S2_row[:], in_=pm2[:])
                t2_row = rtmp.tile([1, TC], F32, name="t2", tag="t_row")
                nc.vector.tensor_tensor(out=t2_row[:], in0=S2_row[:],
                                        in1=S2_row[:], op=OP.mult)
                var2_row = rtmp.tile([1, TC], F32, name="var2",
                                     tag="var_row")
                nc.vector.scalar_tensor_tensor(
                    out=var2_row[:], in0=t2_row[:], scalar=-1.0 / 512.0,
                    in1=ps2[:], op0=OP.mult, op1=OP.add)
                ln2_row = rtmp.tile([1, TC], F32, name="ln2", tag="ln_row")
                nc.scalar.activation(out=ln2_row[:], in_=var2_row[:],
                                     func=AF.Ln, scale=1.0 / 512.0)
                rstd2_row = rows.tile([1, TC], F32, name="rstd2_row")
                nc.scalar.activation(out=rstd2_row[:], in_=ln2_row[:],
                                     func=AF.Exp, scale=-0.5)
                mr_row = rows.tile([1, TC], F32, name="mr_row")
                nc.vector.tensor_tensor(out=mr_row[:], in0=S2_row[:],
                                        in1=rstd2_row[:], op=OP.mult)
                pmrep = prepC.tile([P, 512], F32, name="pmrep", tag="rep")
                nc.tensor.matmul(pmrep[:], oned_row[:],
                                 mr_row.bitcast(F32R)[:],
                                 start=True, stop=True)
                prrep2 = prepC.tile([P, 512], F32, name="prrep2", tag="rep")
                nc.tensor.matmul(prrep2[:], ones_row[:],
                                 rstd2_row.bitcast(F32R)[:],
                                 start=True, stop=True)

                z28 = phc.tile([P, KC, TC], FP8, name="z28")
                for m in range(KC):
                    a = rtmp.tile([P, TC], BF16, name="a_z",
                                  tag=f"az_{m % 2}")
                    with nc.allow_low_precision(reason="bf16/fp8 z2"):
                        nc.vector.tensor_tensor(out=a[:], in0=y2T[:, m, :],
                                                in1=prrep2[:], op=OP.mult)
                        nc.gpsimd.tensor_tensor(out=z28[:, m, :], in0=a[:],
                                                in1=pmrep[:],
                                                op=OP.subtract)
                pstatC_stack.close()

                # MLP
                h18 = phc.tile([P, MC1, TC], FP8, name="h18")
                ph2_stack = ExitStack()
                ph2 = ph2_stack.enter_context(
                    tc.tile_pool(name="ph2", bufs=1, space="PSUM"))
                p2s = [ph2.tile([P, 512], F32, name=f"p2_{m}", tag=f"p2_{m}")
                       for m in range(KC)]
                for k in range(MC1):
                    ks = slice(k * P, k * P + P)
                    p1 = pmmC.tile([P, 512], F32, name="p1", tag="po")
                    for t in range(2):
                        nc.tensor.matmul(p1[:], W1_s[:, 2 * t:2 * t + 2, ks],
                                         z28[:, 2 * t:2 * t + 2, :],
                                         start=(t == 0), stop=(t == 1),
                                         perf_mode=DR)
                    with nc.allow_low_precision(reason="fp8 gelu"):
                        nc.scalar.activation(out=h18[:, k, :], in_=p1[:],
                                             func=AF.Gelu, scale=0.125,
                                             bias=ccol_sb[:, 4 + k:5 + k])
                    if k % 2 == 1:
                        kp = k // 2
                        for m in range(KC):
                            ms = slice(m * P, m * P + P)
                            nc.tensor.matmul(
                                p2s[m][:],
                                W2_s[:, 2 * kp:2 * kp + 2, ms],
                                h18[:, 2 * kp:2 * kp + 2, :],
                                start=(kp == 0), stop=(kp == MC1 // 2 - 1),
                                perf_mode=DR)

                out_sb = phc.tile([P, KC, TC], F32, name="out_sb")
                outT_r = outT.rearrange("(o p) t -> p o t", p=P)
                out_engs = [nc.sync, nc.gpsimd, nc.sync, nc.gpsimd]
                for m in range(KC):
                    nc.vector.scalar_tensor_tensor(
                        out=out_sb[:, m, :], in0=p2s[m][:],
                        scalar=ccol_sb[:, 12 + m:13 + m],
                        in1=y2T[:, m, :], op0=OP.add, op1=OP.add)
                    out_engs[m].dma_start(out=outT_r[:, m, :],
                                          in_=out_sb[:, m, :])
                ph2_stack.close()

    return _finalize(nc) if do_finalize else nc


def prep_inputs(y, Wq, bq, Wk, bk, Wv, bv, Wo, bo, ln1_g, ln1_b, ln2_g, ln2_b,
                W1, b1, W2, b2):
    """Host-side weight folding + fp8 quantization + per-core input maps."""
    f = np.float32
    f8 = ml_dtypes.float8_e4m3
    Wq_ = (Wq * ln1_g[:, None]).astype(f)
    Wk_ = (Wk * ln1_g[:, None]).astype(f)
    Wv_ = (Wv * ln1_g[:, None]).astype(f)
    bq_ = (ln1_b @ Wq + bq).astype(f)
    bv_ = (ln1_b @ Wv + bv).astype(f)
    bo_ = (bv_ @ Wo + bo).astype(f)
    W1_ = (W1 * ln2_g[:, None]).astype(f)
    b1_ = (ln2_b @ W1 + b1).astype(f)

    # crow rows are scaled x8 (matching the x8 weight quantization) and
    # by 1/512 where they multiply S = sum(x) rather than mean(x).
    crow = np.stack([
        8.0 * bq_,
        -8.0 * Wq_.sum(0) / 512.0,
        -8.0 * Wk_.sum(0) / 512.0,
        -8.0 * Wv_.sum(0) / 512.0,
    ]).astype(f)
    ccol = np.concatenate([
        np.zeros((P, 4), f), b1_.reshape(8, P).T,
        np.asarray(b2, f).reshape(4, P).T,
    ], axis=1).astype(f)

    shared = {
        "Wq8": np.ascontiguousarray((8.0 * Wq_).astype(f8)),
        "Wk8": np.ascontiguousarray((8.0 * Wk_).astype(f8)),
        "Wv8": np.ascontiguousarray((8.0 * Wv_).astype(f8)),
        "Wo8": np.ascontiguousarray(np.asarray(Wo, f).astype(f8)),
        "W18": np.ascontiguousarray((8.0 * W1_).astype(f8)),
        "W28": np.ascontiguousarray(np.asarray(W2, f).astype(f8)),
        "crow": crow, "ccol": ccol,
    }
    in_maps = []
    for c in range(8):
        b, s = divmod(c, 4)
        ts = s * TC
        yTm = np.asarray(y, f)[b].T
        yrot = np.ascontiguousarray(np.roll(yTm, -ts, axis=1))
        in_maps.append({
            "y8T": yrot.astype(f8),
            "yoT": np.ascontiguousarray(yrot[:, 0:TC] + bo_[:, None]),
            **shared,
        })
    return in_maps


def gather_output(results):
    out = np.empty((B, N, D), np.float32)
    for c in range(8):
        b, s = divmod(c, 4)
        out[b, s * TC:(s + 1) * TC, :] = results[c]["outT"].T
    return out


_NC_CACHE = {}


def kernel(**inputs):
    """Full-input entry point: shard, run on 8 NeuronCores, gather."""
    from concourse.bass_utils import run_bass_kernel_spmd

    in_maps = prep_inputs(**{k: np.asarray(v) for k, v in inputs.items()})
    if "nc" not in _NC_CACHE:
        _NC_CACHE["nc"] = build_nc()
    nc = _NC_CACHE["nc"]
    res = run_bass_kernel_spmd(nc, in_maps, core_ids=list(range(8)))
    return gather_output(res.results)
